# revision 18
# baseline (speedup 1.0000x reference)
"""JointBetaCVAE forward — Bass/Tile kernel for 8 TRN2 NeuronCores.

Contract: kernel(**inputs) takes FULL unsharded inputs (keys as in
setup_inputs()) and returns (means, logs, zs), each [16384, 8] f32.

Sharding: data-parallel over the B=256 scenes — 32 scenes per core,
params replicated, no collectives.  Per-core kernel:

  phase 1  social attention pooling  (all-pairs tanh-attention, P=64)
  phase 2  z-attention scores/alphas (same structure, causal mask)
  phase 3  64-step autoregressive VAE sampling (partitions = scenes)

Layouts (per core, b = 32 scenes, groups g of 2 scenes):
  big tanh tensors: [(s,h)=128 part, (row,col) free] built by DVE
    stride-0 outer-sum, tanh on ACT, h-reduction via PE matmul with a
    block-diagonal Wf, scores DMA'd into [(s,row)=128, col=64] for
    softmax.
  phase 3: partitions = scenes(32); zacc = sum_{p<j} a2[j,p] (z_p@W1d)
    via per-scene PE block matmuls (T=8) for past blocks + DVE
    scalar_tensor_tensor FMAs in-block.
"""

import hashlib
import os
import tempfile
import threading

import numpy as np

B, P, H, ND, ATT = 256, 64, 64, 8, 64
N = B * P
NCORES = 8
BS = B // NCORES          # 32 scenes per core
ROWS = BS * P             # 2048 rows per core
NG = BS // 2              # 16 two-scene groups
TBLK = 8                  # phase-3 block size

_lock = threading.Lock()
_cached = {}              # key -> (nc, names)


# ---------------------------------------------------------------------------
# host-side parameter prep
# ---------------------------------------------------------------------------

def _prep_consts(p, np_f32=np.float32):
    f32 = lambda a: np.ascontiguousarray(a, np_f32)
    # [128, 32] block-diagonal Wf; cols 2..31 zero so each score matmul
    # initializes a full 32-aligned PSUM partition block
    Wf2_x = np.zeros((128, 32), np.float32)
    Wf2_x[:64, 0] = p["Wf_x"][:, 0]
    Wf2_x[64:, 1] = p["Wf_x"][:, 0]
    Wf2_z = np.zeros((128, 32), np.float32)
    Wf2_z[:64, 0] = p["Wf_z"][:, 0]
    Wf2_z[64:, 1] = p["Wf_z"][:, 0]
    W1 = p["W1"]
    bias_x = (p["be_x"] + p["bl_x"] + p["bc_x"]).reshape(64, 1)
    bias_z = (p["be_z"] + p["bl_z"]).reshape(64, 1)
    jj, pp = np.meshgrid(np.arange(P), np.arange(P), indexing="ij")
    tri = (pp < jj).astype(np.float32)              # [j, p]
    consts = {
        "We_x": f32(p["We_x"]), "Wc_x": f32(p["Wc_x"]),
        "Wl_x": f32(p["Wl_x"]), "Wl_xn": f32(-p["Wl_x"]),
        "We_z": f32(p["We_z"]),
        "Wl_z": f32(p["Wl_z"]), "Wl_zn": f32(-p["Wl_z"]),
        "Wf2_x": Wf2_x, "Wf2_z": Wf2_z,
        "W1a": f32(W1[0:64]), "W1b": f32(W1[64:128]),
        "W1c": f32(W1[128:192]), "W1d": f32(W1[192:200]),
        "b1r": f32(p["b1"].reshape(1, 128)),
        "W2m": f32(p["W2"][:, 0:8]), "W2l": f32(p["W2"][:, 8:16]),
        "b2m": f32(p["b2"][0:8].reshape(8, 1)),
        "b2l": f32(p["b2"][8:16].reshape(8, 1)),
        "b2lh": f32(0.5 * p["b2"][8:16].reshape(8, 1)),
        "bias_x2": f32(np.tile(bias_x, (2, 1))),
        "bias_z2": f32(np.tile(bias_z, (2, 1))),
        "tri2": f32(np.tile(tri, (2, 1))),
        "ident": f32(np.eye(128)),
        "ones1": f32(np.ones((1, 64))),
    }
    return consts


# ---------------------------------------------------------------------------
# Bass program
# ---------------------------------------------------------------------------

def _build_nc(consts):
    import concourse.bass as bass
    import concourse.mybir as mybir
    from concourse import bacc, tile

    dt = mybir.dt
    Alu = mybir.AluOpType
    Act = mybir.ActivationFunctionType

    nc = bacc.Bacc()
    f32 = dt.float32
    bf16 = dt.bfloat16

    dram_in = {}
    for nm, shp in [("xe", [ROWS, 64]), ("xl", [ROWS, 2]), ("eps", [ROWS, 8])]:
        dram_in[nm] = nc.declare_dram_parameter(nm, shp, f32, isOutput=False)
    for nm, arr in consts.items():
        d = bf16 if nm in ("Wf2_x", "Wf2_z") else f32
        dram_in[nm] = nc.declare_dram_parameter(nm, list(arr.shape), d,
                                                isOutput=False)
    d_means = nc.declare_dram_parameter("means8", [8, ROWS], f32, isOutput=True)
    d_logs = nc.declare_dram_parameter("logs8", [8, ROWS], f32, isOutput=True)
    d_zs = nc.declare_dram_parameter("zs8", [8, ROWS], f32, isOutput=True)

    A = bass.AP  # manual access patterns: A(handle, elem_offset, [[stride, n], ...])

    with tile.TileContext(nc) as tc:
        import contextlib
        ctx = contextlib.ExitStack()
        with ctx:
            pc = ctx.enter_context(tc.tile_pool(name="pc", bufs=1))   # params
            pg = ctx.enter_context(tc.tile_pool(name="pg", bufs=2))   # group tiles
            pbig = ctx.enter_context(tc.tile_pool(name="pbig", bufs=3))
            pper = ctx.enter_context(tc.tile_pool(name="pper", bufs=1))  # persistent
            import contextlib as _ctl
            ctx12 = _ctl.ExitStack()
            ps12 = ctx12.enter_context(
                tc.tile_pool(name="ps12", bufs=6, space="PSUM"))

            # ---- load params to SBUF ----
            sb = {}
            for nm in consts:
                d = bf16 if nm in ("Wf2_x", "Wf2_z") else f32
                t = pc.tile(list(consts[nm].shape), d, tag=f"c_{nm}")
                nc.sync.dma_start(out=t[:], in_=dram_in[nm][:])
                sb[nm] = t

            ident = sb["ident"]

            # ---- persistent intermediates ----
            G_all = pper.tile([32, P * 128], f32, tag="G_all")      # [b,(j,h1)]
            A3 = pper.tile([32, P * P], f32, tag="A3")              # [b,(j,p)]
            AT_all = pper.tile([64, BS * P], f32, tag="AT")         # [p,(b,j)]
            ZWT = pper.tile([64, BS * 128], f32, tag="ZWT")         # [p,(b,h1)]
            epsT = pper.tile([8, ROWS], f32, tag="epsT")            # [d,(b,j)]
            o_mean = pper.tile([8, ROWS], f32, tag="o_mean")        # [d,(j,b)]
            o_log = pper.tile([8, ROWS], f32, tag="o_log")
            o_zs = pper.tile([8, ROWS], f32, tag="o_zs")

            NCH = 8         # score chunks per group (8 rows each)
            CH = P // NCH   # 8 rows per chunk

            def attn_scores(g, xeT, xlT, W_e, W_l, W_ln, W_c, bias2, Wf2,
                            score_all):
                """u = W_e@xeT (+W_l@xlT), v = (W_c@xeT) - W_l@xlT.
                score_all [(s,row),col] <- sum_h Wf[h] tanh(u[h,col]+v[h,row]).
                Phase1: row=i (v from W_c), col=j. Phase2: row=j (v=-lz,
                W_c=None), col=p."""
                up = ps12.tile([128, 64], f32, tag="ps")
                vp = ps12.tile([128, 64], f32, tag="ps")
                for s in (0, 1):
                    rhs_e = xeT[:, 64 * s:64 * s + 64]
                    rhs_l = xlT[:, 64 * s:64 * s + 64]
                    nc.tensor.matmul(up[64 * s:64 * s + 64, :], sb[W_e][:],
                                     rhs_e, start=True, stop=False)
                    nc.tensor.matmul(up[64 * s:64 * s + 64, :], sb[W_l][:],
                                     rhs_l, start=False, stop=True)
                    if W_c is not None:
                        nc.tensor.matmul(vp[64 * s:64 * s + 64, :], sb[W_c][:],
                                         rhs_e, start=True, stop=False)
                        nc.tensor.matmul(vp[64 * s:64 * s + 64, :], sb[W_ln][:],
                                         rhs_l, start=False, stop=True)
                    else:
                        nc.tensor.matmul(vp[64 * s:64 * s + 64, :], sb[W_ln][:],
                                         rhs_l, start=True, stop=True)
                u_r = pg.tile([128, 64], f32, tag="u_r")
                v_r = pg.tile([128, 64], f32, tag="v_r")
                # copy psum->sbuf, folding the tanh bias into u
                nc.scalar.activation(u_r[:], up[:], Act.Identity,
                                     bias=sb[bias2][:])
                nc.scalar.activation(v_r[:], vp[:], Act.Identity)
                # 8 chunks of 8 rows; 4 chunks packed per PSUM bank-tile at
                # partition bases {0,32,64,96} (PE out must be 32-aligned)
                for t in range(2):
                    scp = ps12.tile([128, CH * 64], f32, tag="ps")
                    for cc in range(4):
                        c = 4 * t + cc
                        tin = pbig.tile([128, CH * 64], bf16, tag="tin")
                        in0 = A(v_r.tensor, c * CH, [[64, 128], [1, CH], [0, 64]])
                        in1 = A(u_r.tensor, 0, [[64, 128], [0, CH], [1, 64]])
                        nc.vector.tensor_tensor(out=tin[:], in0=in0, in1=in1,
                                                op=Alu.add)
                        tt = pbig.tile([128, CH * 64], bf16, tag="tt")
                        nc.scalar.activation(tt[:], tin[:], Act.Tanh)
                        nc.tensor.matmul(scp[32 * cc:32 * cc + 32, :],
                                         sb[Wf2][:], tt[:], start=True,
                                         stop=True, tile_position=(0, 32 * cc))
                    scs = pg.tile([128, CH * 64], f32, tag="scs")
                    if t == 0:
                        nc.scalar.activation(scs[:], scp[:], Act.Identity)
                    else:
                        nc.vector.tensor_copy(out=scs[:], in_=scp[:])
                    for cc in range(4):
                        c = 4 * t + cc
                        for s in (0, 1):
                            nc.sync.dma_start(
                                out=score_all[64 * s + c * CH:
                                              64 * s + (c + 1) * CH, :],
                                in_=scs[32 * cc + s:32 * cc + s + 1, :])

            for g in range(NG):
                r0 = g * 128
                # ---- loads & transposes ----
                xe2 = pg.tile([128, 64], f32, tag="xe2")
                nc.sync.dma_start(out=xe2[:], in_=dram_in["xe"][r0:r0 + 128, :])
                xl2 = pg.tile([128, 2], f32, tag="xl2")
                nc.sync.dma_start(out=xl2[:], in_=dram_in["xl"][r0:r0 + 128, :])
                xeTp = ps12.tile([64, 128], f32, tag="ps")
                nc.tensor.transpose(xeTp[:], xe2[:], ident[:])
                xeT = pg.tile([64, 128], f32, tag="xeTs")
                nc.scalar.activation(xeT[:], xeTp[:], Act.Identity)
                xlTp = ps12.tile([2, 128], f32, tag="ps")
                nc.tensor.transpose(xlTp[:], xl2[:], ident[:])
                xlT = pg.tile([2, 128], f32, tag="xlTs")
                nc.scalar.activation(xlT[:], xlTp[:], Act.Identity)
                xe_s = []
                for s in (0, 1):
                    t = pg.tile([64, 64], f32, tag=f"xe_s{s}")
                    nc.sync.dma_start(
                        out=t[:], in_=dram_in["xe"][r0 + 64 * s:r0 + 64 * s + 64, :])
                    xe_s.append(t)

                # ================= phase 1: social =================
                sc1 = pg.tile([128, 64], f32, tag="sc1")
                attn_scores(g, xeT, xlT, "We_x", "Wl_x", "Wl_xn", "Wc_x",
                            "bias_x2", "Wf2_x", sc1)
                # softmax over j (cols), mask all-true
                mx = pg.tile([128, 1], f32, tag="mx1")
                nc.vector.tensor_reduce(mx[:], sc1[:], mybir.AxisListType.X,
                                        Alu.max, negate=True)
                alpha = pg.tile([128, 64], f32, tag="alpha")
                den = pg.tile([128, 1], f32, tag="den1")
                nc.scalar.activation(alpha[:], sc1[:], Act.Exp, bias=mx[:],
                                     accum_out=den[:])
                rec = pg.tile([128, 1], f32, tag="rec1")
                nc.vector.reciprocal(rec[:], den[:])
                nc.vector.tensor_scalar(out=alpha[:], in0=alpha[:],
                                        scalar1=rec[:], scalar2=None,
                                        op0=Alu.mult)
                aTp = ps12.tile([64, 128], f32, tag="ps")
                nc.tensor.transpose(aTp[:], alpha[:], ident[:])
                aT = pg.tile([64, 128], f32, tag="aTs")
                nc.scalar.activation(aT[:], aTp[:], Act.Identity)

                # ================= phase 2: z-attention =================
                sc2 = pg.tile([128, 64], f32, tag="sc2")
                attn_scores(g, xeT, xlT, "We_z", "Wl_z", "Wl_zn", None,
                            "bias_z2", "Wf2_z", sc2)
                ms = pg.tile([128, 64], f32, tag="ms")
                nc.vector.tensor_tensor(out=ms[:], in0=sc2[:], in1=sb["tri2"][:],
                                        op=Alu.mult)
                mx2 = pg.tile([128, 1], f32, tag="mx2")
                nc.vector.tensor_reduce(mx2[:], ms[:], mybir.AxisListType.X,
                                        Alu.max, negate=True)
                e2 = pg.tile([128, 64], f32, tag="e2")
                nc.scalar.activation(e2[:], ms[:], Act.Exp, bias=mx2[:])
                nc.vector.tensor_tensor(out=e2[:], in0=e2[:], in1=sb["tri2"][:],
                                        op=Alu.mult)
                den2 = pg.tile([128, 1], f32, tag="den2")
                nc.vector.tensor_reduce(den2[:], e2[:], mybir.AxisListType.X,
                                        Alu.add)
                rec2 = pg.tile([128, 1], f32, tag="rec2")
                nc.vector.tensor_scalar(out=den2[:], in0=den2[:], scalar1=1e-10,
                                        scalar2=None, op0=Alu.add)
                nc.vector.reciprocal(rec2[:], den2[:])
                a2 = pg.tile([128, 64], f32, tag="a2")
                nc.vector.tensor_scalar(out=a2[:], in0=e2[:], scalar1=rec2[:],
                                        scalar2=None, op0=Alu.mult)
                # A3[b,(j,p)]
                nc.sync.dma_start(out=A3[2 * g:2 * g + 2, :], in_=a2[:])
                # AT_all[p,(b,j)]
                a2Tp = ps12.tile([64, 128], f32, tag="ps")
                nc.tensor.transpose(a2Tp[:], a2[:], ident[:])
                nc.scalar.activation(AT_all[:, 128 * g:128 * g + 128], a2Tp[:],
                                     Act.Identity)

                # ---- socialT, z_prev_xT, G ----
                for s in (0, 1):
                    soTp = ps12.tile([64, 64], f32, tag="ps")
                    nc.tensor.matmul(soTp[:], xe_s[s][:],
                                     aT[:, 64 * s:64 * s + 64],
                                     start=True, stop=True)
                    soT = pg.tile([64, 64], f32, tag=f"soT{s}")
                    nc.scalar.activation(soT[:], soTp[:], Act.Identity)
                    zxTp = ps12.tile([64, 64], f32, tag="ps")
                    nc.tensor.matmul(zxTp[:], xe_s[s][:],
                                     AT_all[:, 128 * g + 64 * s:128 * g + 64 * s + 64],
                                     start=True, stop=True)
                    zxT = pg.tile([64, 64], f32, tag=f"zxT{s}")
                    nc.scalar.activation(zxT[:], zxTp[:], Act.Identity)
                    Gp = ps12.tile([64, 128], f32, tag="ps")
                    nc.tensor.matmul(Gp[:], xeT[:, 64 * s:64 * s + 64],
                                     sb["W1a"][:], start=True, stop=False)
                    nc.tensor.matmul(Gp[:], soT[:], sb["W1b"][:],
                                     start=False, stop=False)
                    nc.tensor.matmul(Gp[:], zxT[:], sb["W1c"][:],
                                     start=False, stop=False)
                    nc.tensor.matmul(Gp[:], sb["ones1"][:], sb["b1r"][:],
                                     start=False, stop=True)
                    Gsb = pg.tile([64, 128], f32, tag="Gsb")
                    nc.scalar.activation(Gsb[:], Gp[:], Act.Identity)
                    nc.sync.dma_start(out=G_all[2 * g + s:2 * g + s + 1, :],
                                      in_=Gsb[:])

                # ---- eps transpose: epsT[d, 64b+j] ----
                epn = pg.tile([128, 8], f32, tag="epn")
                nc.sync.dma_start(out=epn[:], in_=dram_in["eps"][r0:r0 + 128, :])
                epp = ps12.tile([8, 128], f32, tag="ps")
                nc.tensor.transpose(epp[:], epn[:], ident[:])
                nc.scalar.activation(epsT[:, 128 * g:128 * g + 128], epp[:],
                                     Act.Identity)

            ctx12.close()

            # ================= phase 3: recurrence =================
            ps3 = ctx.enter_context(
                tc.tile_pool(name="ps3", bufs=2, space="PSUM"))

            for k in range(P // TBLK):
                t0 = k * TBLK
                if k > 0:
                    # zacc_past[j in block] = sum_{p<t0} a2[j,p] zw[p]
                    # per scene: zaccT [128h, 8j] = ZWT_slice.T @ AT_slice;
                    # 16 scenes gathered into zaT [128h, 128(b',j)], PE-
                    # transposed to [(b',j), h], then row-ified into zp.
                    zp = pg.tile([32, TBLK * 128], f32, tag="zp")
                    for x in (0, 1):
                        zaT = pg.tile([128, 128], f32, tag="zaT")
                        for bb in range(16):
                            b = 16 * x + bb
                            lhsT = A(ZWT.tensor, 128 * b,
                                     [[BS * 128, t0], [1, 128]])
                            rhs = A(AT_all.tensor, 64 * b + t0,
                                    [[BS * P, t0], [1, TBLK]])
                            zaTp = ps3.tile([128, TBLK], f32, tag="h1T")
                            nc.tensor.matmul(zaTp[:], lhsT, rhs,
                                             start=True, stop=True)
                            nc.scalar.activation(
                                zaT[:, TBLK * bb:TBLK * bb + TBLK],
                                zaTp[:], Act.Identity)
                        ztp = ps3.tile([128, 128], f32, tag="zw")
                        nc.tensor.transpose(ztp[:], zaT[:], ident[:])
                        zts = pg.tile([128, 128], f32, tag="zts")
                        nc.scalar.activation(zts[:], ztp[:], Act.Identity)
                        nc.sync.dma_start(
                            out=zp[16 * x:16 * x + 16, :], in_=zts[:])
                    Gz = pg.tile([32, TBLK * 128], f32, tag="Gz")
                    gsl = A(G_all.tensor, 1024 * k, [[P * 128, 32], [1, 1024]])
                    nc.vector.tensor_tensor(out=Gz[:], in0=zp[:], in1=gsl,
                                            op=Alu.add)
                    Gz_t = Gz.tensor
                    gz_off = 0
                    gz_pstride = TBLK * 128
                else:
                    Gz_t = G_all.tensor
                    gz_off = 0
                    gz_pstride = P * 128

                zwblk = pg.tile([32, TBLK * 128], f32, tag="zwblk")
                for jj in range(TBLK):
                    j = t0 + jj
                    gz_j = A(Gz_t, gz_off + 128 * jj, [[gz_pstride, 32], [1, 128]])
                    acc = None
                    for p in range(t0, j):
                        scl = A(A3.tensor, 64 * j + p, [[P * P, 32], [1, 1]])
                        if p == j - 1:
                            zw_src = zwp_prev[:]
                        else:
                            zw_src = A(zwblk.tensor, 128 * (p - t0),
                                       [[TBLK * 128, 32], [1, 128]])
                        in1 = gz_j if acc is None else acc[:]
                        nacc = pg.tile([32, 128], f32, tag="acc")
                        nc.vector.scalar_tensor_tensor(
                            out=nacc[:], in0=zw_src, scalar=scl, in1=in1,
                            op0=Alu.mult, op1=Alu.add)
                        acc = nacc
                    relu_in = gz_j if acc is None else acc[:]
                    h1 = pg.tile([32, 128], f32, tag="h1")
                    nc.scalar.activation(h1[:], relu_in, Act.Relu)
                    h1Tp = ps3.tile([128, 32], f32, tag="h1T")
                    nc.tensor.transpose(h1Tp[:], h1[:], ident[0:32, 0:32])
                    h1T = pg.tile([128, 32], f32, tag="h1Ts")
                    nc.scalar.activation(h1T[:], h1Tp[:], Act.Identity)
                    om = ps3.tile([8, 32], f32, tag="oml")
                    ol = ps3.tile([8, 32], f32, tag="oml")
                    nc.tensor.matmul(om[:], sb["W2m"][:], h1T[:],
                                     start=True, stop=True)
                    nc.tensor.matmul(ol[:], sb["W2l"][:], h1T[:],
                                     start=True, stop=True)
                    E = pg.tile([8, 32], f32, tag="E")
                    nc.scalar.activation(E[:], ol[:], Act.Exp, scale=0.5,
                                         bias=sb["b2lh"][:])
                    tzz = pg.tile([8, 32], f32, tag="tzz")
                    eps_j = A(epsT.tensor, j, [[ROWS, 8], [64, 32]])
                    nc.vector.tensor_tensor(out=tzz[:], in0=E[:], in1=eps_j,
                                            op=Alu.mult)
                    z_j = A(o_zs.tensor, 32 * j, [[ROWS, 8], [1, 32]])
                    nc.vector.scalar_tensor_tensor(
                        out=z_j, in0=om[:], scalar=sb["b2m"][:], in1=tzz[:],
                        op0=Alu.add, op1=Alu.add)
                    # mean = z - eps*std, computed from SBUF so it can go on
                    # the otherwise-idle GPSIMD engine (it cannot read PSUM)
                    m_j = A(o_mean.tensor, 32 * j, [[ROWS, 8], [1, 32]])
                    nc.gpsimd.tensor_tensor(out=m_j, in0=z_j, in1=tzz[:],
                                            op=Alu.subtract)
                    l_j = A(o_log.tensor, 32 * j, [[ROWS, 8], [1, 32]])
                    nc.vector.tensor_scalar(out=l_j, in0=ol[:],
                                            scalar1=sb["b2l"][:], scalar2=None,
                                            op0=Alu.add)
                    zwp = ps3.tile([32, 128], f32, tag="zw")
                    nc.tensor.matmul(zwp[:], z_j, sb["W1d"][:],
                                     start=True, stop=True)
                    zwp_prev = zwp
                    zwb_j = A(zwblk.tensor, 128 * jj, [[TBLK * 128, 32], [1, 128]])
                    nc.scalar.activation(zwb_j, zwp[:], Act.Identity)
                    if k < P // TBLK - 1:
                        nc.sync.dma_start(out=ZWT[j:j + 1, :], in_=zwb_j)

            # ---- outputs ----
            nc.sync.dma_start(out=d_means[:], in_=o_mean[:])
            nc.sync.dma_start(out=d_logs[:], in_=o_log[:])
            nc.sync.dma_start(out=d_zs[:], in_=o_zs[:])

    nc.finalize()
    return nc


# ---------------------------------------------------------------------------
# compile cache: wrap compile_bir_kernel with a disk cache so a fresh
# process skips the multi-minute walrus compile
# ---------------------------------------------------------------------------

_CACHE_DIR = os.environ.get("BASSK_NEFF_CACHE",
                            os.path.expanduser("~/.bassk_neff_cache"))


def _install_neff_cache():
    from concourse import bass2jax
    if getattr(bass2jax, "_bassk_cache_installed", False):
        return
    orig = bass2jax.compile_bir_kernel

    def cached_compile(bir_json, tmpdir, neff_name="file.neff"):
        key = hashlib.sha256(
            bir_json if isinstance(bir_json, bytes) else bir_json.encode()
        ).hexdigest()
        os.makedirs(_CACHE_DIR, exist_ok=True)
        path = os.path.join(_CACHE_DIR, key + ".neff")
        if os.path.exists(path):
            out = os.path.join(tmpdir, neff_name)
            with open(path, "rb") as f, open(out, "wb") as g:
                g.write(f.read())
            return out
        neff = orig(bir_json, tmpdir, neff_name)
        tmp = path + f".tmp{os.getpid()}"
        with open(neff, "rb") as f, open(tmp, "wb") as g:
            g.write(f.read())
        os.replace(tmp, path)
        return neff

    bass2jax.compile_bir_kernel = cached_compile
    bass2jax._bassk_cache_installed = True


# ---------------------------------------------------------------------------
# public entry point
# ---------------------------------------------------------------------------

def _prepare(x_enc, x_last, eps, params):
    consts = _prep_consts(params)
    x_enc = np.ascontiguousarray(x_enc, np.float32)
    x_last = np.ascontiguousarray(x_last, np.float32)
    eps_a = np.ascontiguousarray(eps, np.float32)
    _install_neff_cache()
    with _lock:
        if "nc" not in _cached:
            _cached["nc"] = _build_nc(consts)
        nc = _cached["nc"]
    import ml_dtypes
    in_maps = []
    for c in range(NCORES):
        r = slice(c * ROWS, (c + 1) * ROWS)
        m = {"xe": x_enc[r], "xl": x_last[r], "eps": eps_a[r]}
        for nm, arr in consts.items():
            if nm in ("Wf2_x", "Wf2_z"):
                m[nm] = arr.astype(ml_dtypes.bfloat16)
            else:
                m[nm] = arr
        in_maps.append(m)
    return nc, in_maps


def _gather(res):
    means = np.empty((B, P, ND), np.float32)
    logs = np.empty((B, P, ND), np.float32)
    zs = np.empty((B, P, ND), np.float32)
    for c in range(NCORES):
        r = res.results[c]
        # device layout [8, (j, b)]: col = 32*j + b
        for name, dst in (("means8", means), ("logs8", logs), ("zs8", zs)):
            a = np.asarray(r[name]).reshape(8, P, BS)        # [d, j, b]
            dst[c * BS:(c + 1) * BS] = a.transpose(2, 1, 0)  # [b, j, d]
    unp = lambda t: t.reshape(N, ND)
    return unp(means), unp(logs), unp(zs)


def _params_dict(We_x, be_x, Wl_x, bl_x, Wc_x, bc_x, Wf_x, bf_x,
                 We_z, be_z, Wl_z, bl_z, Wf_z, bf_z, W1, b1, W2, b2):
    f = lambda a: np.asarray(a, np.float32)
    return dict(We_x=f(We_x), be_x=f(be_x), Wl_x=f(Wl_x), bl_x=f(bl_x),
                Wc_x=f(Wc_x), bc_x=f(bc_x), Wf_x=f(Wf_x), bf_x=f(bf_x),
                We_z=f(We_z), be_z=f(be_z), Wl_z=f(Wl_z), bl_z=f(bl_z),
                Wf_z=f(Wf_z), bf_z=f(bf_z), W1=f(W1), b1=f(b1),
                W2=f(W2), b2=f(b2))


def kernel(x_enc, x_last, seq_start_end, eps,
           We_x, be_x, Wl_x, bl_x, Wc_x, bc_x, Wf_x, bf_x,
           We_z, be_z, Wl_z, bl_z, Wf_z, bf_z,
           W1, b1, W2, b2):
    params = _params_dict(We_x, be_x, Wl_x, bl_x, Wc_x, bc_x, Wf_x, bf_x,
                          We_z, be_z, Wl_z, bl_z, Wf_z, bf_z, W1, b1, W2, b2)
    from concourse.bass_utils import run_bass_kernel_spmd
    nc, in_maps = _prepare(x_enc, x_last, eps, params)
    res = run_bass_kernel_spmd(nc, in_maps, core_ids=list(range(NCORES)))
    return _gather(res)


def run_profiled(x_enc, x_last, seq_start_end, eps,
                 We_x, be_x, Wl_x, bl_x, Wc_x, bc_x, Wf_x, bf_x,
                 We_z, be_z, Wl_z, bl_z, Wf_z, bf_z,
                 W1, b1, W2, b2, tmpdir=None):
    """Run with NTFF tracing; returns profiled exec_time_ns (or None)."""
    params = _params_dict(We_x, be_x, Wl_x, bl_x, Wc_x, bc_x, Wf_x, bf_x,
                          We_z, be_z, Wl_z, bl_z, Wf_z, bf_z, W1, b1, W2, b2)
    from concourse.bass_utils import run_bass_kernel_spmd
    nc, in_maps = _prepare(x_enc, x_last, eps, params)
    res = run_bass_kernel_spmd(nc, in_maps, core_ids=list(range(NCORES)),
                               trace=True, tmpdir=tmpdir)
    globals()["_last_profile"] = res
    return res.exec_time_ns
